# revision 14
# baseline (speedup 1.0000x reference)
"""Trainium2 Bass kernel for nn_MessageLayer (GNN message passing), 8 NeuronCores.

Reference computation:
    edge_mat = (edge_features @ W + b).reshape(E, 64, 16)
    messages = einsum('emh,eh->em', edge_mat, hidden[edge_sources])
    out      = segment_sum(messages, edge_targets, num_segments=10000)

Algebraic restructure (cuts FLOPs 32x): since aggregation is linear,
    out[n, m] = sum_{f,h} W[f, m*16+h] * C[n, f, h],
    C[n, f, h] = sum_{e: tgt(e)=n} ef[e, f] * hidden[src(e), h]

Structure (v2.3): per-target segments ("positions", split at 64) are packed
into full-array K=128 matmuls in two species:
  - BIG (33..64 edges): 2 row-slots of 64 x 4 ef column-classes
    = 8 positions/matmul, moving [128, 128]
  - SMALL (<=32 edges): 4 row-slots of 32 x 4 classes
    = 16 positions/matmul, moving [128, 256]
Stationary [128, 128]: row r of slot j holds the 4 class-edges' features at
column groups 32g..32g+32 (dense).  Moving: slot j's rows carry the 4
source-hidden vectors at cols 64j+16g+h, zeros elsewhere (slot separation;
zeros memset on-device, data DMA'd compactly per slot-band).
PSUM out: valid C-blocks at (32g+f, stripe 16s+h) with s%4 == g uniformly
across both species, garbage elsewhere.  Each bank (4 big or 2 small
matmuls) drains as two half-width [128, 256] f32->bf16 copies (DVE + ACT in
parallel) into the spread c_spread.
W-stage: 4 concurrent 32-row-strip matmul chains (one per class g), each
reading its valid columns via stride-64:  c_spread[32g:32g+32, (16g+h)::64]
-> [32, U], against a 4x-replicated W stationary [32f@32g, 64m] (both
m-halves at once, 16 accumulating h-phases into po_g [64, U] PSUM).

Sharding: node-ownership (scatter-reduce by target): core c owns nodes
[1250c, 1250c+1250) and receives exactly the edges targeting them, so no
collective is needed; host assembles per-position rows into final output.
All tensors bf16 on the wire/SBUF (f32 PSUM accumulate): rel-err ~3.5e-3
vs the 2e-2 gate.
"""
import numpy as np
from contextlib import ExitStack

N_NODES = 10000
N_EDGES = 320000
HID = 16
MSG = 64
EFD = 32
NCORES = 8
NPC = N_NODES // NCORES          # 1250 nodes owned per core
CPBUFS = 4                       # PSUM tiles for C banks (4 + 4 po = 8)

_CACHE = {}


def _bf16():
    import ml_dtypes
    return ml_dtypes.bfloat16


def _build_layout(edge_targets):
    """Per-core position lists (node, edge-ids, len<=64, sorted desc; all
    len>32 "big" positions precede the "small" ones) plus the SPMD-uniform
    grid: T_big 8-position matmuls then T_small 16-position matmuls."""
    segs_per_core, nbig_per_core = [], []
    for c in range(NCORES):
        lo = c * NPC
        mask = (edge_targets >= lo) & (edge_targets < lo + NPC)
        eids = np.nonzero(mask)[0]
        tgt = edge_targets[eids]
        order = np.argsort(tgt, kind="stable")
        eids = eids[order]
        tgt = tgt[order]
        segs = []
        uniq, starts = np.unique(tgt, return_index=True)
        bounds = list(starts) + [len(tgt)]
        for i, n in enumerate(uniq):
            s, e = bounds[i], bounds[i + 1]
            while e - s > 64:
                segs.append((int(n), eids[s:s + 64]))
                s += 64
            segs.append((int(n), eids[s:e]))
        segs.sort(key=lambda t: -len(t[1]))
        segs_per_core.append(segs)
        nbig_per_core.append(sum(1 for _, e in segs if len(e) > 32))

    T_big = -(-max(nbig_per_core) // 8)
    T_big = ((T_big + 7) // 8) * 8            # whole double-banks of 8 matmuls
    nsmall = max(len(s) - b for s, b in zip(segs_per_core, nbig_per_core))
    T_small = -(-nsmall // 16)
    T_small = ((T_small + 3) // 4) * 4        # whole double-banks of 4 matmuls
    U = 2 * T_big + 4 * T_small               # total position quads
    assert U <= 512, f"U={U} exceeds one PSUM bank"
    return segs_per_core, nbig_per_core, T_big, T_small, U


def _build_w2(W):
    # w2[32g+f, 64h+m] = W[f, m*16+h], replicated across the 4 class groups
    Wr = W.reshape(EFD, MSG, HID).transpose(0, 2, 1)   # [f, h, m]
    blk = np.ascontiguousarray(Wr.reshape(EFD, HID * MSG))
    return np.tile(blk, (4, 1)).astype(np.float32)     # [128, 1024]


def _pack_core(segs, nbig, T_big, T_small, w2, edge_features, edge_sources,
               hidden):
    """DRAM image per core, bf16:
      [128, T*128 st | T_big*64 mv-big | T_small*64 mv-small | 1024 w2]
    BIG position p<8*T_big (t=p//8, j=(p//4)%2, g=p%4):
      st[64j+r, t*128+32g+f];  mv-big band j at partitions 64j: [64j+r, t*64+16g+h]
    SMALL position q (t=q//16, j=(q//4)%4, g=q%4):
      st[32j+r, (T_big+t)*128+32g+f];  mv-small band j at partitions 32j."""
    T = T_big + T_small
    St = np.zeros((128, T * 128), dtype=np.float32)
    MvB = np.zeros((128, T_big * 64), dtype=np.float32)
    MvS = np.zeros((128, T_small * 64), dtype=np.float32)
    for i in range(len(segs)):
        _, eids = segs[i]
        k = len(eids)
        if i < nbig:
            t, j, g = i // 8, (i // 4) % 2, i % 4
            r0 = 64 * j
            St[r0:r0 + k, t * 128 + 32 * g:t * 128 + 32 * g + EFD] = \
                edge_features[eids]
            MvB[r0:r0 + k, t * 64 + 16 * g:t * 64 + 16 * g + HID] = \
                hidden[edge_sources[eids]]
        else:
            q = i - nbig
            t, j, g = q // 16, (q // 4) % 4, q % 4
            r0 = 32 * j
            St[r0:r0 + k, (T_big + t) * 128 + 32 * g:
               (T_big + t) * 128 + 32 * g + EFD] = edge_features[eids]
            MvS[r0:r0 + k, t * 64 + 16 * g:t * 64 + 16 * g + HID] = \
                hidden[edge_sources[eids]]
    D = np.concatenate([St, MvB, MvS, w2], axis=1)
    return np.ascontiguousarray(D.astype(_bf16()))


def _chunks(T, n, align):
    bs = [((round(k * T / n)) // align) * align for k in range(n)] + [T]
    bs[1] = max(bs[1], align) if T >= align else bs[1]
    return [(bs[k], bs[k + 1]) for k in range(n) if bs[k + 1] > bs[k]]


def _build_program(T_big, T_small, U):
    import concourse.tile as tile
    from concourse import bacc, mybir

    f32 = mybir.dt.float32
    bf16 = mybir.dt.bfloat16
    T = T_big + T_small
    ST_W = T * 128
    DB_big = T_big // 8                  # double-bank fills (1024 f32 cols)
    DB = DB_big + T_small // 4
    MVB_SB = T_big * 128                 # big region width in mv_sb

    nc = bacc.Bacc("TRN2", target_bir_lowering=False, debug=False,
                   num_devices=NCORES)
    data_dram = nc.dram_tensor(
        "data", [128, ST_W + (T_big + T_small) * 64 + 1024], bf16,
        kind="ExternalInput").ap()
    out_dram = nc.dram_tensor("out", [128, 2 * U], bf16,
                              kind="ExternalOutput").ap()

    NB = 16                              # mv ring depth (tiles) per species
    CH = 8                               # DMA chunk size (tiles)

    with tile.TileContext(nc) as tc, ExitStack() as ctx:
        big = ctx.enter_context(tc.tile_pool(name="big", bufs=1))
        cpool = ctx.enter_context(tc.tile_pool(name="cps", bufs=3,
                                               space="PSUM"))
        opool = ctx.enter_context(tc.tile_pool(name="ops", bufs=1,
                                               space="PSUM"))

        st_sb = big.tile([128, ST_W], bf16, tag="st")
        # mv RING buffers, band-major: the slot-separation zeros are
        # memset ONCE (engine memset cost scales with column count, so a
        # full-width zeroed region is unaffordable); band DMAs then cycle
        # through the ring touching only their own partition rows, leaving
        # the complement rows zero forever.  WAR deps (ring reuse) resolve
        # at matmul pace, which always outruns the DMA.
        rb_sb = big.tile([128, 2 * NB * 64], bf16, tag="rb")
        rs_sb = big.tile([128, 4 * NB * 64], bf16, tag="rs")
        w2_sb = big.tile([128, 1024], bf16, tag="w2")
        c_spread = big.tile([128, DB * 1024], bf16, tag="csp")
        out_sb = big.tile([128, 2 * U], bf16, tag="outsb")
        wu_sb = big.tile([128, 64], bf16, tag="wu")

        # PE warm-up with full-128-row matmuls (1-row matmuls do NOT
        # register as PE-busy for the HAM clock gate): ~3.5us of sustained
        # activity un-throttles the PE clock 1.2 -> 2.4 GHz.
        nc.vector.memset(wu_sb[:], 0.25)
        wups = cpool.tile([128, 1024], f32, tag="cps", name="wups")
        for _ in range(60):
            nc.tensor.matmul(wups[0:64, 0:64], wu_sb[:, 0:64],
                             wu_sb[:, 0:64], start=True, stop=True)

        # one-time ring zeros, one op per band region so each band's first
        # DMA waits only on its own region's memset
        for j in range(2):
            nc.vector.memset(rb_sb[:, j * NB * 64:(j + 1) * NB * 64], 0.0)
        for j in range(4):
            nc.gpsimd.memset(rs_sb[:, j * NB * 64:(j + 1) * NB * 64], 0.0)
        # w2 rides the otherwise-idle SWDGE queue, off the two HWDGE queues
        nc.gpsimd.dma_start(w2_sb[:],
                            data_dram[:, ST_W + (T_big + T_small) * 64:])

        q = [nc.sync, nc.scalar]
        nbig_ch = -(-T_big // CH)
        nsml_ch = -(-T_small // CH)

        # C stage; whole-double-bank drains (1024 f32 cols) amortize the
        # per-op overhead; alternate DVE/ACT 4:3 (DVE is faster)
        di = [0]

        def drain(ps, db):
            eng = nc.vector.tensor_copy if di[0] % 7 < 4 else nc.scalar.copy
            di[0] += 1
            eng(c_spread[:, db * 1024:(db + 1) * 1024], ps[:])

        # W stage: po_g[m, u] += sum_f W[f, m*16+h] * C[u, g, f, h]
        # 4 strips on 4 distinct row groups; strips (0,1)->poA, (2,3)->poB
        # packed into col groups 0/64 of the same PSUM bank.  Split into
        # two u-halves: half 1 is issued mid-C-stage (its banks are
        # already drained) so it runs while the PE would otherwise wait
        # on DMA; only half 2 remains as tail work.
        poA = opool.tile([128, 512], f32, tag="poA", name="poA")[:, 0:U]
        poB = opool.tile([128, 512], f32, tag="poB", name="poB")[:, 0:U]
        po = [poA, poA, poB, poB]
        DB1 = (DB * 5) // 8              # u-split point (double-banks)
        U1 = DB1 * 16                    # 16 64-col quads per double-bank

        def w_stage(u0, u1):
            csl = c_spread[:, u0 * 64:u1 * 64]
            for h in range(HID):
                for g in range(4):   # g inner: 4 strips run concurrently
                    nc.tensor.matmul(
                        po[g][64 * (g % 2):64 * (g % 2) + MSG, u0:u1],
                        w2_sb[32 * g:32 * g + 32, 64 * h:64 * h + 64],
                        csl[32 * g:32 * g + 32, (16 * g + h)::64],
                        start=(h == 0), stop=(h == HID - 1),
                        skip_group_check=True,
                        tile_position=(32 * g, 64 * (g % 2)))

        mv_big = rb_sb.rearrange("p (r c) -> p r c", r=2)
        mv_small = rs_sb.rearrange("p (r c) -> p r c", r=4)

        # Tile does NOT insert DMA-write-after-matmul-read WAR deps, so
        # ring-slot reuse needs explicit edges: the chunk-k mv DMA depends
        # on the last matmul that read those ring slots (chunk k-2).
        from concourse.tile import add_dep_helper
        mm_big, mm_small = [], []

        def dma_big_chunk(k):
            b0, b1 = k * CH, min((k + 1) * CH, T_big)
            r0, r1 = (b0 % NB) * 64, (b0 % NB) * 64 + (b1 - b0) * 64
            q[k % 2].dma_start(st_sb[:, b0 * 128:b1 * 128],
                               data_dram[:, b0 * 128:b1 * 128])
            for j in range(2):
                d = q[(k + 1) % 2].dma_start(
                    rb_sb[64 * j:64 * j + 64,
                          j * NB * 64 + r0:j * NB * 64 + r1],
                    data_dram[64 * j:64 * j + 64,
                              ST_W + b0 * 64:ST_W + b1 * 64])
                if b0 >= NB:
                    add_dep_helper(d.ins, mm_big[b1 - NB - 1].ins,
                                   reason="mv ring WAR")

        def dma_small_chunk(k):
            s0, s1 = k * CH, min((k + 1) * CH, T_small)
            r0, r1 = (s0 % NB) * 64, (s0 % NB) * 64 + (s1 - s0) * 64
            kk = k + nbig_ch
            q[kk % 2].dma_start(
                st_sb[:, (T_big + s0) * 128:(T_big + s1) * 128],
                data_dram[:, (T_big + s0) * 128:(T_big + s1) * 128])
            for j in range(4):
                d = q[(kk + 1) % 2].dma_start(
                    rs_sb[32 * j:32 * j + 32,
                          j * NB * 64 + r0:j * NB * 64 + r1],
                    data_dram[32 * j:32 * j + 32,
                              ST_W + T_big * 64 + s0 * 64:
                              ST_W + T_big * 64 + s1 * 64])
                if s0 >= NB:
                    add_dep_helper(d.ins, mm_small[s1 - NB - 1].ins,
                                   reason="mv ring WAR")

        ps = None

        def mm_big_tile(t):
            nonlocal ps
            if t % 8 == 0:
                ps = cpool.tile([128, 1024], f32, tag="cps")
            tr = t % NB
            mm_big.append(nc.tensor.matmul(
                ps[:, 128 * (t % 8):128 * (t % 8) + 128],
                st_sb[:, t * 128:(t + 1) * 128],
                mv_big[:, :, tr * 64:(tr + 1) * 64],
                start=True, stop=True))
            if t % 8 == 7:
                drain(ps, t // 8)
                if t // 8 + 1 == DB1:
                    w_stage(0, U1)   # overlapped W half-1 (PE is DMA-gated)

        def mm_small_tile(ts):
            nonlocal ps
            if ts % 4 == 0:
                ps = cpool.tile([128, 1024], f32, tag="cps")
            tr = ts % NB
            mm_small.append(nc.tensor.matmul(
                ps[:, 256 * (ts % 4):256 * (ts % 4) + 256],
                st_sb[:, (T_big + ts) * 128:(T_big + ts + 1) * 128],
                mv_small[:, :, tr * 64:(tr + 1) * 64],
                start=True, stop=True))
            if ts % 4 == 3:
                drain(ps, DB_big + ts // 4)
                if DB_big + ts // 4 + 1 == DB1:
                    w_stage(0, U1)   # overlapped W half-1 (PE is DMA-gated)

        # chunk-pipelined issue: DMA chunk k goes out one iteration before
        # its matmuls, so WAR targets (chunk k-2's matmuls) already exist
        for k in range(nbig_ch + 1):
            if k < nbig_ch:
                dma_big_chunk(k)
            if k >= 1:
                for t in range((k - 1) * CH, min(k * CH, T_big)):
                    mm_big_tile(t)
        for k in range(nsml_ch + 1):
            if k < nsml_ch:
                dma_small_chunk(k)
            if k >= 1:
                for ts in range((k - 1) * CH, min(k * CH, T_small)):
                    mm_small_tile(ts)
        w_stage(U1, U)
        nc.vector.tensor_copy(out_sb[:, 0:U], poA[:])
        nc.scalar.copy(out_sb[:, U:2 * U], poB[:])
        nc.sync.dma_start(out_dram[:], out_sb[:])
    nc.compile()
    return nc


def _assemble(outs, segs_per_core, nbig_per_core, T_big, U):
    out = np.zeros((N_NODES, MSG), dtype=np.float32)
    mrow = np.arange(MSG)[None, :]
    for c in range(NCORES):
        segs = segs_per_core[c]
        nbig = nbig_per_core[c]
        P = len(segs)
        if P == 0:
            continue
        po_sb = outs[c].astype(np.float32)           # [128, 2U]
        i = np.arange(P)
        p = np.where(i < nbig, i, 8 * T_big + (i - nbig))  # grid position
        u, g = p // 4, p % 4
        part = 64 * (g % 2)[:, None] + mrow          # [P, 64]
        col = ((g // 2) * U + u)[:, None]
        pos_rows = po_sb[part, col]                  # [P, 64]
        nodes = np.fromiter((segs[k][0] for k in range(P)), dtype=np.int64,
                            count=P)
        np.add.at(out, nodes, pos_rows)
    return out


def kernel(node_features, edge_features, edge_sources, edge_targets,
           hidden, initial, W, b):
    from concourse.bass_utils import run_bass_kernel_spmd

    edge_targets = np.asarray(edge_targets)
    edge_sources = np.asarray(edge_sources)
    edge_features = np.asarray(edge_features, dtype=np.float32)
    hidden = np.asarray(hidden, dtype=np.float32)
    W = np.asarray(W, dtype=np.float32)
    b = np.asarray(b, dtype=np.float32)

    key = edge_targets.tobytes()
    if key in _CACHE:
        layout, nc = _CACHE[key]
    else:
        layout = _build_layout(edge_targets)
        segs_per_core, nbig_per_core, T_big, T_small, U = layout
        nc = _build_program(T_big, T_small, U)
        _CACHE[key] = (layout, nc)
    segs_per_core, nbig_per_core, T_big, T_small, U = layout

    w2 = _build_w2(W)
    in_maps = []
    for c in range(NCORES):
        data = _pack_core(segs_per_core[c], nbig_per_core[c], T_big, T_small,
                          w2, edge_features, edge_sources, hidden)
        in_maps.append({"data": data})

    res = run_bass_kernel_spmd(nc, in_maps, list(range(NCORES)))
    outs = [res.results[c]["out"] for c in range(NCORES)]
    out = _assemble(outs, segs_per_core, nbig_per_core, T_big, U)

    if np.any(b):
        # bias term: out[n] += (sum_{e->n} hidden[src e]) @ Br,
        # Br[h, m] = b[m*16+h].  (b is all-zero for this problem.)
        Br = b.reshape(MSG, HID).T.astype(np.float32)
        acc = np.zeros((N_NODES, HID), dtype=np.float32)
        np.add.at(acc, edge_targets, hidden[edge_sources])
        out += acc @ Br
    return out



# revision 15
# speedup vs baseline: 1.0715x; 1.0715x over previous
"""Trainium2 Bass kernel for nn_MessageLayer (GNN message passing), 8 NeuronCores.

Reference computation:
    edge_mat = (edge_features @ W + b).reshape(E, 64, 16)
    messages = einsum('emh,eh->em', edge_mat, hidden[edge_sources])
    out      = segment_sum(messages, edge_targets, num_segments=10000)

Algebraic restructure (cuts FLOPs 32x): since aggregation is linear,
    out[n, m] = sum_{f,h} W[f, m*16+h] * C[n, f, h],
    C[n, f, h] = sum_{e: tgt(e)=n} ef[e, f] * hidden[src(e), h]

Structure (v2.3): per-target segments ("positions", split at 64) are packed
into full-array K=128 matmuls in two species:
  - BIG (33..64 edges): 2 row-slots of 64 x 4 ef column-classes
    = 8 positions/matmul, moving [128, 128]
  - SMALL (<=32 edges): 4 row-slots of 32 x 4 classes
    = 16 positions/matmul, moving [128, 256]
Stationary [128, 128]: row r of slot j holds the 4 class-edges' features at
column groups 32g..32g+32 (dense).  Moving: slot j's rows carry the 4
source-hidden vectors at cols 64j+16g+h, zeros elsewhere (slot separation;
zeros memset on-device, data DMA'd compactly per slot-band).
PSUM out: valid C-blocks at (32g+f, stripe 16s+h) with s%4 == g uniformly
across both species, garbage elsewhere.  Each bank (4 big or 2 small
matmuls) drains as two half-width [128, 256] f32->bf16 copies (DVE + ACT in
parallel) into the spread c_spread.
W-stage: 4 concurrent 32-row-strip matmul chains (one per class g), each
reading its valid columns via stride-64:  c_spread[32g:32g+32, (16g+h)::64]
-> [32, U], against a 4x-replicated W stationary [32f@32g, 64m] (both
m-halves at once, 16 accumulating h-phases into po_g [64, U] PSUM).

Sharding: node-ownership (scatter-reduce by target): core c owns nodes
[1250c, 1250c+1250) and receives exactly the edges targeting them, so no
collective is needed; host assembles per-position rows into final output.
All tensors bf16 on the wire/SBUF (f32 PSUM accumulate): rel-err ~3.5e-3
vs the 2e-2 gate.
"""
import numpy as np
from contextlib import ExitStack

N_NODES = 10000
N_EDGES = 320000
HID = 16
MSG = 64
EFD = 32
NCORES = 8
NPC = N_NODES // NCORES          # 1250 nodes owned per core
CPBUFS = 4                       # PSUM tiles for C banks (4 + 4 po = 8)

_CACHE = {}


def _bf16():
    import ml_dtypes
    return ml_dtypes.bfloat16


def _build_layout(edge_targets):
    """Per-core position lists (node, edge-ids, len<=64, sorted desc; all
    len>32 "big" positions precede the "small" ones) plus the SPMD-uniform
    grid: T_big 8-position matmuls then T_small 16-position matmuls."""
    segs_per_core, nbig_per_core = [], []
    for c in range(NCORES):
        lo = c * NPC
        mask = (edge_targets >= lo) & (edge_targets < lo + NPC)
        eids = np.nonzero(mask)[0]
        tgt = edge_targets[eids]
        order = np.argsort(tgt, kind="stable")
        eids = eids[order]
        tgt = tgt[order]
        segs = []
        uniq, starts = np.unique(tgt, return_index=True)
        bounds = list(starts) + [len(tgt)]
        for i, n in enumerate(uniq):
            s, e = bounds[i], bounds[i + 1]
            while e - s > 64:
                segs.append((int(n), eids[s:s + 64]))
                s += 64
            segs.append((int(n), eids[s:e]))
        segs.sort(key=lambda t: -len(t[1]))
        segs_per_core.append(segs)
        nbig_per_core.append(sum(1 for _, e in segs if len(e) > 32))

    T_big = -(-max(nbig_per_core) // 8)
    T_big = ((T_big + 7) // 8) * 8            # whole double-banks of 8 matmuls
    nsmall = max(len(s) - b for s, b in zip(segs_per_core, nbig_per_core))
    T_small = -(-nsmall // 16)
    T_small = ((T_small + 3) // 4) * 4        # whole double-banks of 4 matmuls
    U = 2 * T_big + 4 * T_small               # total position quads
    assert U <= 512, f"U={U} exceeds one PSUM bank"
    return segs_per_core, nbig_per_core, T_big, T_small, U


def _build_w2(W):
    # w2[32g+f, 64h+m] = W[f, m*16+h], replicated across the 4 class groups
    Wr = W.reshape(EFD, MSG, HID).transpose(0, 2, 1)   # [f, h, m]
    blk = np.ascontiguousarray(Wr.reshape(EFD, HID * MSG))
    return np.tile(blk, (4, 1)).astype(np.float32)     # [128, 1024]


def _pack_core(segs, nbig, T_big, T_small, w2, edge_features, edge_sources,
               hidden):
    """DRAM image per core, bf16:
      [128, T*128 st | T_big*64 mv-big | T_small*64 mv-small | 1024 w2]
    BIG position p<8*T_big (t=p//8, j=(p//4)%2, g=p%4):
      st[64j+r, t*128+32g+f];  mv-big band j at partitions 64j: [64j+r, t*64+16g+h]
    SMALL position q (t=q//16, j=(q//4)%4, g=q%4):
      st[32j+r, (T_big+t)*128+32g+f];  mv-small band j at partitions 32j."""
    T = T_big + T_small
    St = np.zeros((128, T * 128), dtype=np.float32)
    MvB = np.zeros((128, T_big * 64), dtype=np.float32)
    MvS = np.zeros((128, T_small * 64), dtype=np.float32)
    for i in range(len(segs)):
        _, eids = segs[i]
        k = len(eids)
        if i < nbig:
            t, j, g = i // 8, (i // 4) % 2, i % 4
            r0 = 64 * j
            St[r0:r0 + k, t * 128 + 32 * g:t * 128 + 32 * g + EFD] = \
                edge_features[eids]
            MvB[r0:r0 + k, t * 64 + 16 * g:t * 64 + 16 * g + HID] = \
                hidden[edge_sources[eids]]
        else:
            q = i - nbig
            t, j, g = q // 16, (q // 4) % 4, q % 4
            r0 = 32 * j
            St[r0:r0 + k, (T_big + t) * 128 + 32 * g:
               (T_big + t) * 128 + 32 * g + EFD] = edge_features[eids]
            MvS[r0:r0 + k, t * 64 + 16 * g:t * 64 + 16 * g + HID] = \
                hidden[edge_sources[eids]]
    D = np.concatenate([St, MvB, MvS, w2], axis=1)
    return np.ascontiguousarray(D.astype(_bf16()))


def _chunks(T, n, align):
    bs = [((round(k * T / n)) // align) * align for k in range(n)] + [T]
    bs[1] = max(bs[1], align) if T >= align else bs[1]
    return [(bs[k], bs[k + 1]) for k in range(n) if bs[k + 1] > bs[k]]


def _build_program(T_big, T_small, U):
    import concourse.tile as tile
    from concourse import bacc, mybir

    f32 = mybir.dt.float32
    bf16 = mybir.dt.bfloat16
    T = T_big + T_small
    ST_W = T * 128
    DB_big = T_big // 8                  # double-bank fills (1024 f32 cols)
    DB = DB_big + T_small // 4
    MVB_SB = T_big * 128                 # big region width in mv_sb

    nc = bacc.Bacc("TRN2", target_bir_lowering=False, debug=False,
                   num_devices=NCORES)
    data_dram = nc.dram_tensor(
        "data", [128, ST_W + (T_big + T_small) * 64 + 1024], bf16,
        kind="ExternalInput").ap()
    out_dram = nc.dram_tensor("out", [128, 2 * U], bf16,
                              kind="ExternalOutput").ap()

    NB = 24                              # mv ring depth (tiles) per species
    CH = 8                               # DMA chunk size (tiles)

    with tile.TileContext(nc) as tc, ExitStack() as ctx:
        big = ctx.enter_context(tc.tile_pool(name="big", bufs=1))
        cpool = ctx.enter_context(tc.tile_pool(name="cps", bufs=3,
                                               space="PSUM"))
        opool = ctx.enter_context(tc.tile_pool(name="ops", bufs=1,
                                               space="PSUM"))

        st_sb = big.tile([128, ST_W], bf16, tag="st")
        # mv RING buffers, band-major: the slot-separation zeros are
        # memset ONCE (engine memset cost scales with column count, so a
        # full-width zeroed region is unaffordable); band DMAs then cycle
        # through the ring touching only their own partition rows, leaving
        # the complement rows zero forever.  WAR deps (ring reuse) resolve
        # at matmul pace, which always outruns the DMA.
        rb_sb = big.tile([128, 2 * NB * 64], bf16, tag="rb")
        rs_sb = big.tile([128, 4 * NB * 64], bf16, tag="rs")
        w2_sb = big.tile([128, 1024], bf16, tag="w2")
        c_spread = big.tile([128, DB * 1024], bf16, tag="csp")
        out_sb = big.tile([128, 2 * U], bf16, tag="outsb")
        wu_sb = big.tile([128, 64], bf16, tag="wu")

        # PE warm-up with full-128-row matmuls (1-row matmuls do NOT
        # register as PE-busy for the HAM clock gate): ~3.5us of sustained
        # activity un-throttles the PE clock 1.2 -> 2.4 GHz.
        nc.vector.memset(wu_sb[:], 0.25)
        wups = cpool.tile([128, 1024], f32, tag="cps", name="wups")
        for _ in range(60):
            nc.tensor.matmul(wups[0:64, 0:64], wu_sb[:, 0:64],
                             wu_sb[:, 0:64], start=True, stop=True)

        # one-time ring zeros, one op per band region so each band's first
        # DMA waits only on its own region's memset
        for j in range(2):
            nc.vector.memset(rb_sb[:, j * NB * 64:(j + 1) * NB * 64], 0.0)
        for j in range(4):
            if j % 2 == 0:
                nc.gpsimd.memset(rs_sb[:, j * NB * 64:(j + 1) * NB * 64],
                                 0.0)
            else:
                nc.scalar.memzero(rs_sb[:, j * NB * 64:(j + 1) * NB * 64])
        # w2 rides the otherwise-idle SWDGE queue, off the two HWDGE queues
        nc.gpsimd.dma_start(w2_sb[:],
                            data_dram[:, ST_W + (T_big + T_small) * 64:])

        q = [nc.sync, nc.scalar]
        nbig_ch = -(-T_big // CH)
        nsml_ch = -(-T_small // CH)

        # C stage; whole-double-bank drains (1024 f32 cols) amortize the
        # per-op overhead; alternate DVE/ACT 4:3 (DVE is faster)
        di = [0]

        def drain(ps, db):
            eng = nc.vector.tensor_copy if di[0] % 2 == 0 else nc.scalar.copy
            di[0] += 1
            eng(c_spread[:, db * 1024:(db + 1) * 1024], ps[:])

        # W stage: po_g[m, u] += sum_f W[f, m*16+h] * C[u, g, f, h]
        # 4 strips on 4 distinct row groups; strips (0,1)->poA, (2,3)->poB
        # packed into col groups 0/64 of the same PSUM bank.  Split into
        # two u-halves: half 1 is issued mid-C-stage (its banks are
        # already drained) so it runs while the PE would otherwise wait
        # on DMA; only half 2 remains as tail work.
        poA = opool.tile([128, 512], f32, tag="poA", name="poA")[:, 0:U]
        poB = opool.tile([128, 512], f32, tag="poB", name="poB")[:, 0:U]
        po = [poA, poA, poB, poB]
        DB1 = DB_big                     # u-split: whole big species
        U1 = DB1 * 16                    # 16 64-col quads per double-bank

        def w_stage(u0, u1):
            csl = c_spread[:, u0 * 64:u1 * 64]
            for h in range(HID):
                for g in range(4):   # g inner: 4 strips run concurrently
                    nc.tensor.matmul(
                        po[g][64 * (g % 2):64 * (g % 2) + MSG, u0:u1],
                        w2_sb[32 * g:32 * g + 32, 64 * h:64 * h + 64],
                        csl[32 * g:32 * g + 32, (16 * g + h)::64],
                        start=(h == 0), stop=(h == HID - 1),
                        skip_group_check=True,
                        tile_position=(32 * g, 64 * (g % 2)))

        mv_big = rb_sb.rearrange("p (r c) -> p r c", r=2)
        mv_small = rs_sb.rearrange("p (r c) -> p r c", r=4)

        # Tile does NOT insert DMA-write-after-matmul-read WAR deps, so
        # ring-slot reuse needs explicit edges: the chunk-k mv DMA depends
        # on the last matmul that read those ring slots (chunk k-2).
        from concourse.tile import add_dep_helper
        mm_big, mm_small = [], []

        def dma_big_chunk(k):
            b0, b1 = k * CH, min((k + 1) * CH, T_big)
            r0, r1 = (b0 % NB) * 64, (b0 % NB) * 64 + (b1 - b0) * 64
            q[k % 2].dma_start(st_sb[:, b0 * 128:b1 * 128],
                               data_dram[:, b0 * 128:b1 * 128])
            for j in range(2):
                d = q[(k + 1) % 2].dma_start(
                    rb_sb[64 * j:64 * j + 64,
                          j * NB * 64 + r0:j * NB * 64 + r1],
                    data_dram[64 * j:64 * j + 64,
                              ST_W + b0 * 64:ST_W + b1 * 64])
                if b0 >= NB:
                    add_dep_helper(d.ins, mm_big[b1 - NB - 1].ins,
                                   reason="mv ring WAR")

        def dma_small_chunk(k):
            s0, s1 = k * CH, min((k + 1) * CH, T_small)
            r0, r1 = (s0 % NB) * 64, (s0 % NB) * 64 + (s1 - s0) * 64
            kk = k + nbig_ch
            q[kk % 2].dma_start(
                st_sb[:, (T_big + s0) * 128:(T_big + s1) * 128],
                data_dram[:, (T_big + s0) * 128:(T_big + s1) * 128])
            for j in range(4):
                d = q[(kk + 1) % 2].dma_start(
                    rs_sb[32 * j:32 * j + 32,
                          j * NB * 64 + r0:j * NB * 64 + r1],
                    data_dram[32 * j:32 * j + 32,
                              ST_W + T_big * 64 + s0 * 64:
                              ST_W + T_big * 64 + s1 * 64])
                if s0 >= NB:
                    add_dep_helper(d.ins, mm_small[s1 - NB - 1].ins,
                                   reason="mv ring WAR")

        ps = None

        def mm_big_tile(t):
            nonlocal ps
            if t % 8 == 0:
                ps = cpool.tile([128, 1024], f32, tag="cps")
            tr = t % NB
            mm_big.append(nc.tensor.matmul(
                ps[:, 128 * (t % 8):128 * (t % 8) + 128],
                st_sb[:, t * 128:(t + 1) * 128],
                mv_big[:, :, tr * 64:(tr + 1) * 64],
                start=True, stop=True))
            if t % 8 == 7:
                drain(ps, t // 8)

        def mm_small_tile(ts):
            nonlocal ps
            if ts % 4 == 0:
                ps = cpool.tile([128, 1024], f32, tag="cps")
            tr = ts % NB
            mm_small.append(nc.tensor.matmul(
                ps[:, 256 * (ts % 4):256 * (ts % 4) + 256],
                st_sb[:, (T_big + ts) * 128:(T_big + ts + 1) * 128],
                mv_small[:, :, tr * 64:(tr + 1) * 64],
                start=True, stop=True))
            if ts % 4 == 3:
                drain(ps, DB_big + ts // 4)

        # chunk-pipelined issue: DMA chunk k goes out one iteration before
        # its matmuls, so WAR targets (chunk k-2's matmuls) already exist
        for k in range(nbig_ch + 1):
            if k < nbig_ch:
                dma_big_chunk(k)
            if k >= 1:
                for t in range((k - 1) * CH, min(k * CH, T_big)):
                    mm_big_tile(t)
        for k in range(nsml_ch + 1):
            if k < nsml_ch:
                dma_small_chunk(k)
            if k == 1:
                w_stage(0, U1)   # W half-1 while small DMAs stream
            if k >= 1:
                for ts in range((k - 1) * CH, min(k * CH, T_small)):
                    mm_small_tile(ts)
        w_stage(U1, U)
        nc.vector.tensor_copy(out_sb[:, 0:U], poA[:])
        nc.scalar.copy(out_sb[:, U:2 * U], poB[:])
        nc.sync.dma_start(out_dram[:], out_sb[:])
    nc.compile()
    return nc


def _assemble(outs, segs_per_core, nbig_per_core, T_big, U):
    out = np.zeros((N_NODES, MSG), dtype=np.float32)
    mrow = np.arange(MSG)[None, :]
    for c in range(NCORES):
        segs = segs_per_core[c]
        nbig = nbig_per_core[c]
        P = len(segs)
        if P == 0:
            continue
        po_sb = outs[c].astype(np.float32)           # [128, 2U]
        i = np.arange(P)
        p = np.where(i < nbig, i, 8 * T_big + (i - nbig))  # grid position
        u, g = p // 4, p % 4
        part = 64 * (g % 2)[:, None] + mrow          # [P, 64]
        col = ((g // 2) * U + u)[:, None]
        pos_rows = po_sb[part, col]                  # [P, 64]
        nodes = np.fromiter((segs[k][0] for k in range(P)), dtype=np.int64,
                            count=P)
        np.add.at(out, nodes, pos_rows)
    return out


def kernel(node_features, edge_features, edge_sources, edge_targets,
           hidden, initial, W, b):
    from concourse.bass_utils import run_bass_kernel_spmd

    edge_targets = np.asarray(edge_targets)
    edge_sources = np.asarray(edge_sources)
    edge_features = np.asarray(edge_features, dtype=np.float32)
    hidden = np.asarray(hidden, dtype=np.float32)
    W = np.asarray(W, dtype=np.float32)
    b = np.asarray(b, dtype=np.float32)

    key = edge_targets.tobytes()
    if key in _CACHE:
        layout, nc = _CACHE[key]
    else:
        layout = _build_layout(edge_targets)
        segs_per_core, nbig_per_core, T_big, T_small, U = layout
        nc = _build_program(T_big, T_small, U)
        _CACHE[key] = (layout, nc)
    segs_per_core, nbig_per_core, T_big, T_small, U = layout

    w2 = _build_w2(W)
    in_maps = []
    for c in range(NCORES):
        data = _pack_core(segs_per_core[c], nbig_per_core[c], T_big, T_small,
                          w2, edge_features, edge_sources, hidden)
        in_maps.append({"data": data})

    res = run_bass_kernel_spmd(nc, in_maps, list(range(NCORES)))
    outs = [res.results[c]["out"] for c in range(NCORES)]
    out = _assemble(outs, segs_per_core, nbig_per_core, T_big, U)

    if np.any(b):
        # bias term: out[n] += (sum_{e->n} hidden[src e]) @ Br,
        # Br[h, m] = b[m*16+h].  (b is all-zero for this problem.)
        Br = b.reshape(MSG, HID).T.astype(np.float32)
        acc = np.zeros((N_NODES, HID), dtype=np.float32)
        np.add.at(acc, edge_targets, hidden[edge_sources])
        out += acc @ Br
    return out



# revision 18
# speedup vs baseline: 1.1165x; 1.0420x over previous
"""Trainium2 Bass kernel for nn_MessageLayer (GNN message passing), 8 NeuronCores.

Reference computation:
    edge_mat = (edge_features @ W + b).reshape(E, 64, 16)
    messages = einsum('emh,eh->em', edge_mat, hidden[edge_sources])
    out      = segment_sum(messages, edge_targets, num_segments=10000)

Algebraic restructure (cuts FLOPs 32x): since aggregation is linear,
    out[n, m] = sum_{f,h} W[f, m*16+h] * C[n, f, h],
    C[n, f, h] = sum_{e: tgt(e)=n} ef[e, f] * hidden[src(e), h]

Structure (v2.3): per-target segments ("positions", split at 64) are packed
into full-array K=128 matmuls in two species:
  - BIG (33..64 edges): 2 row-slots of 64 x 4 ef column-classes
    = 8 positions/matmul, moving [128, 128]
  - SMALL (<=32 edges): 4 row-slots of 32 x 4 classes
    = 16 positions/matmul, moving [128, 256]
Stationary [128, 128]: row r of slot j holds the 4 class-edges' features at
column groups 32g..32g+32 (dense).  Moving: slot j's rows carry the 4
source-hidden vectors at cols 64j+16g+h, zeros elsewhere (slot separation;
zeros memset on-device, data DMA'd compactly per slot-band).
PSUM out: valid C-blocks at (32g+f, stripe 16s+h) with s%4 == g uniformly
across both species, garbage elsewhere.  Each bank (4 big or 2 small
matmuls) drains as two half-width [128, 256] f32->bf16 copies (DVE + ACT in
parallel) into the spread c_spread.
W-stage: 4 concurrent 32-row-strip matmul chains (one per class g), each
reading its valid columns via stride-64:  c_spread[32g:32g+32, (16g+h)::64]
-> [32, U], against a 4x-replicated W stationary [32f@32g, 64m] (both
m-halves at once, 16 accumulating h-phases into po_g [64, U] PSUM).

Sharding: node-ownership (scatter-reduce by target): core c owns nodes
[1250c, 1250c+1250) and receives exactly the edges targeting them, so no
collective is needed; host assembles per-position rows into final output.
All tensors bf16 on the wire/SBUF (f32 PSUM accumulate): rel-err ~3.5e-3
vs the 2e-2 gate.
"""
import numpy as np
from contextlib import ExitStack

N_NODES = 10000
N_EDGES = 320000
HID = 16
MSG = 64
EFD = 32
NCORES = 8
NPC = N_NODES // NCORES          # 1250 nodes owned per core
CPBUFS = 4                       # PSUM tiles for C banks (4 + 4 po = 8)

_CACHE = {}


def _bf16():
    import ml_dtypes
    return ml_dtypes.bfloat16


def _build_layout(edge_targets):
    """Per-core position lists (node, edge-ids, len<=64, sorted desc; all
    len>32 "big" positions precede the "small" ones) plus the SPMD-uniform
    grid: T_big 8-position matmuls then T_small 16-position matmuls."""
    segs_per_core, nbig_per_core = [], []
    for c in range(NCORES):
        lo = c * NPC
        mask = (edge_targets >= lo) & (edge_targets < lo + NPC)
        eids = np.nonzero(mask)[0]
        tgt = edge_targets[eids]
        order = np.argsort(tgt, kind="stable")
        eids = eids[order]
        tgt = tgt[order]
        segs = []
        uniq, starts = np.unique(tgt, return_index=True)
        bounds = list(starts) + [len(tgt)]
        for i, n in enumerate(uniq):
            s, e = bounds[i], bounds[i + 1]
            while e - s > 64:
                segs.append((int(n), eids[s:s + 64]))
                s += 64
            segs.append((int(n), eids[s:e]))
        segs.sort(key=lambda t: -len(t[1]))
        segs_per_core.append(segs)
        nbig_per_core.append(sum(1 for _, e in segs if len(e) > 32))

    T_big = -(-max(nbig_per_core) // 8)
    T_big = ((T_big + 7) // 8) * 8            # whole double-banks of 8 matmuls
    nsmall = max(len(s) - b for s, b in zip(segs_per_core, nbig_per_core))
    T_small = -(-nsmall // 16)
    T_small = ((T_small + 3) // 4) * 4        # whole double-banks of 4 matmuls
    # U = po/out column count: 64 u-slots per drained double-bank
    # (big: u = 2t+j, 32 tiles/double-bank; small: u = DB_big*64+4t+j)
    DB_big = -(-T_big // 32)
    DB_small = -(-T_small // 16)
    U = (DB_big + DB_small) * 64
    assert U <= 512, f"U={U} exceeds one PSUM bank"
    return segs_per_core, nbig_per_core, T_big, T_small, U


def _build_w2(W):
    # w2[32g+f, 64h+m] = W[f, m*16+h], replicated across the 4 class groups
    Wr = W.reshape(EFD, MSG, HID).transpose(0, 2, 1)   # [f, h, m]
    blk = np.ascontiguousarray(Wr.reshape(EFD, HID * MSG))
    return np.tile(blk, (4, 1)).astype(np.float32)     # [128, 1024]


def _pack_core(segs, nbig, T_big, T_small, w2, edge_features, edge_sources,
               hidden):
    """DRAM image per core, bf16:
      [128, T*128 st | T_big*64 mv-big | T_small*64 mv-small | 1024 w2]
    BIG position p<8*T_big (t=p//8, j=(p//4)%2, g=p%4):
      st[64j+r, t*128+32g+f];  mv-big band j at partitions 64j: [64j+r, t*64+16g+h]
    SMALL position q (t=q//16, j=(q//4)%4, g=q%4):
      st[32j+r, (T_big+t)*128+32g+f];  mv-small band j at partitions 32j."""
    T = T_big + T_small
    St = np.zeros((128, T * 128), dtype=np.float32)
    MvB = np.zeros((128, T_big * 64), dtype=np.float32)
    MvS = np.zeros((128, T_small * 64), dtype=np.float32)
    for i in range(len(segs)):
        _, eids = segs[i]
        k = len(eids)
        if i < nbig:
            t, j, g = i // 8, (i // 4) % 2, i % 4
            r0 = 64 * j
            St[r0:r0 + k, t * 128 + 32 * g:t * 128 + 32 * g + EFD] = \
                edge_features[eids]
            MvB[r0:r0 + k, t * 64 + 16 * g:t * 64 + 16 * g + HID] = \
                hidden[edge_sources[eids]]
        else:
            q = i - nbig
            t, j, g = q // 16, (q // 4) % 4, q % 4
            r0 = 32 * j
            St[r0:r0 + k, (T_big + t) * 128 + 32 * g:
               (T_big + t) * 128 + 32 * g + EFD] = edge_features[eids]
            MvS[r0:r0 + k, t * 64 + 16 * g:t * 64 + 16 * g + HID] = \
                hidden[edge_sources[eids]]
    D = np.concatenate([St, MvB, MvS, w2], axis=1)
    return np.ascontiguousarray(D.astype(_bf16()))


def _chunks(T, n, align):
    bs = [((round(k * T / n)) // align) * align for k in range(n)] + [T]
    bs[1] = max(bs[1], align) if T >= align else bs[1]
    return [(bs[k], bs[k + 1]) for k in range(n) if bs[k + 1] > bs[k]]


def _build_program(T_big, T_small, U):
    import concourse.tile as tile
    from concourse import bacc, mybir

    f32 = mybir.dt.float32
    bf16 = mybir.dt.bfloat16
    T = T_big + T_small
    ST_W = T * 128
    DB_big = -(-T_big // 32)             # double-bank fills (1024 f32 cols)
    DB = DB_big + -(-T_small // 16)

    nc = bacc.Bacc("TRN2", target_bir_lowering=False, debug=False,
                   num_devices=NCORES)
    data_dram = nc.dram_tensor(
        "data", [128, ST_W + (T_big + T_small) * 64 + 1024], bf16,
        kind="ExternalInput").ap()
    out_dram = nc.dram_tensor("out", [128, 2 * U], bf16,
                              kind="ExternalOutput").ap()

    NB = 24                              # mv ring depth (tiles) per species
    CH = 8                               # DMA chunk size (tiles)

    with tile.TileContext(nc) as tc, ExitStack() as ctx:
        big = ctx.enter_context(tc.tile_pool(name="big", bufs=1))
        cpool = ctx.enter_context(tc.tile_pool(name="cps", bufs=3,
                                               space="PSUM"))
        opool = ctx.enter_context(tc.tile_pool(name="ops", bufs=1,
                                               space="PSUM"))

        st_sb = big.tile([128, ST_W], bf16, tag="st")
        # mv RING buffers, band-major: the slot-separation zeros are
        # memset ONCE (engine memset cost scales with column count, so a
        # full-width zeroed region is unaffordable); band DMAs then cycle
        # through the ring touching only their own partition rows, leaving
        # the complement rows zero forever.  WAR deps (ring reuse) resolve
        # at matmul pace, which always outruns the DMA.
        rb_sb = big.tile([128, 2 * NB * 64], bf16, tag="rb")
        rs_sb = big.tile([128, 4 * NB * 64], bf16, tag="rs")
        w2_sb = big.tile([128, 1024], bf16, tag="w2")
        c_spread = big.tile([128, DB * 1024], bf16, tag="csp")
        out_sb = big.tile([128, 2 * U], bf16, tag="outsb")
        wu_sb = big.tile([128, 64], bf16, tag="wu")

        # PE warm-up with full-128-row matmuls (1-row matmuls do NOT
        # register as PE-busy for the HAM clock gate): ~3.5us of sustained
        # activity un-throttles the PE clock 1.2 -> 2.4 GHz.
        nc.vector.memset(wu_sb[:], 0.25)
        wups = cpool.tile([128, 1024], f32, tag="cps", name="wups")
        for _ in range(60):
            nc.tensor.matmul(wups[0:64, 0:64], wu_sb[:, 0:64],
                             wu_sb[:, 0:64], start=True, stop=True)

        # one-time ring zeros, one op per band region so each band's first
        # DMA waits only on its own region's memset
        for j in range(2):
            nc.vector.memset(rb_sb[:, j * NB * 64:(j + 1) * NB * 64], 0.0)
        for j in range(4):
            if j % 2 == 0:
                nc.gpsimd.memset(rs_sb[:, j * NB * 64:(j + 1) * NB * 64],
                                 0.0)
            else:
                nc.scalar.memzero(rs_sb[:, j * NB * 64:(j + 1) * NB * 64])
        # w2 rides the otherwise-idle SWDGE queue, off the two HWDGE queues
        nc.gpsimd.dma_start(w2_sb[:],
                            data_dram[:, ST_W + (T_big + T_small) * 64:])

        q = [nc.sync, nc.scalar]
        nbig_ch = -(-T_big // CH)
        nsml_ch = -(-T_small // CH)

        # C stage; whole-double-bank drains (1024 f32 cols) amortize the
        # per-op overhead; alternate DVE/ACT 4:3 (DVE is faster)
        di = [0]

        def drain(ps, db, w=1024):
            eng = nc.vector.tensor_copy if di[0] % 2 == 0 else nc.scalar.copy
            di[0] += 1
            eng(c_spread[:, db * 1024:db * 1024 + w], ps[:, 0:w])

        # zero the pad tails of partial double-banks (never matmul-written)
        if T_big * 32 < DB_big * 1024:
            nc.vector.memset(c_spread[:, T_big * 32:DB_big * 1024], 0.0)
        if DB_big * 1024 + T_small * 64 < DB * 1024:
            nc.vector.memset(
                c_spread[:, DB_big * 1024 + T_small * 64:DB * 1024], 0.0)

        # W stage: po_g[m, u] += sum_f W[f, m*16+h] * C[u, g, f, h]
        # 4 strips on 4 distinct row groups; strips (0,1)->poA, (2,3)->poB
        # packed into col groups 0/64 of the same PSUM bank.  Split into
        # two u-halves: half 1 is issued mid-C-stage (its banks are
        # already drained) so it runs while the PE would otherwise wait
        # on DMA; only half 2 remains as tail work.
        poA = opool.tile([128, 512], f32, tag="poA", name="poA")[:, 0:U]
        poB = opool.tile([128, 512], f32, tag="poB", name="poB")[:, 0:U]
        po = [poA, poA, poB, poB]
        DB1 = DB_big                     # u-split: whole big species
        U1 = DB1 * 64                    # 64 u-slots per double-bank

        def w_stage(u0, u1):
            for h in range(HID):
                for g in range(4):   # g inner: 4 strips run concurrently
                    nc.tensor.matmul(
                        po[g][64 * (g % 2):64 * (g % 2) + MSG, u0:u1],
                        w2_sb[32 * g:32 * g + 32, 64 * h:64 * h + 64],
                        c_spread[32 * g:32 * g + 32,
                                 u0 * 16 + h:u1 * 16:16],
                        start=(h == 0), stop=(h == HID - 1),
                        skip_group_check=True,
                        tile_position=(32 * g, 64 * (g % 2)))

        mv_big = rb_sb.rearrange("p (r c) -> p r c", r=2)
        mv_small = rs_sb.rearrange("p (r c) -> p r c", r=4)

        # Tile does NOT insert DMA-write-after-matmul-read WAR deps, so
        # ring-slot reuse needs explicit edges: the chunk-k mv DMA depends
        # on the last matmul that read those ring slots (chunk k-2).
        from concourse.tile import add_dep_helper
        mm_big, mm_small = [], []

        def dma_big_chunk(k):
            b0, b1 = k * CH, min((k + 1) * CH, T_big)
            r0, r1 = (b0 % NB) * 64, (b0 % NB) * 64 + (b1 - b0) * 64
            q[k % 2].dma_start(st_sb[:, b0 * 128:b1 * 128],
                               data_dram[:, b0 * 128:b1 * 128])
            for j in range(2):
                d = q[(k + 1) % 2].dma_start(
                    rb_sb[64 * j:64 * j + 64,
                          j * NB * 64 + r0:j * NB * 64 + r1],
                    data_dram[64 * j:64 * j + 64,
                              ST_W + b0 * 64:ST_W + b1 * 64])
                if b0 >= NB:
                    add_dep_helper(d.ins, mm_big[b1 - NB - 1].ins,
                                   reason="mv ring WAR")

        def dma_small_chunk(k):
            s0, s1 = k * CH, min((k + 1) * CH, T_small)
            r0, r1 = (s0 % NB) * 64, (s0 % NB) * 64 + (s1 - s0) * 64
            kk = k + nbig_ch
            q[kk % 2].dma_start(
                st_sb[:, (T_big + s0) * 128:(T_big + s1) * 128],
                data_dram[:, (T_big + s0) * 128:(T_big + s1) * 128])
            for j in range(4):
                d = q[(kk + 1) % 2].dma_start(
                    rs_sb[32 * j:32 * j + 32,
                          j * NB * 64 + r0:j * NB * 64 + r1],
                    data_dram[32 * j:32 * j + 32,
                              ST_W + T_big * 64 + s0 * 64:
                              ST_W + T_big * 64 + s1 * 64])
                if s0 >= NB:
                    add_dep_helper(d.ins, mm_small[s1 - NB - 1].ins,
                                   reason="mv ring WAR")

        ps = None

        # Each tile runs as 4 concurrent col-group matmuls (one per ef
        # class, tile_position=(0, 32g)): class g writes only partitions
        # [32g, 32g+32), so the four classes stack into the SAME psum
        # columns at different partition bands -- 4x denser PSUM and 4x
        # less drain traffic than the fused [128, N] matmul, with the
        # same stationary/moving data.
        def mm_big_tile(t):
            nonlocal ps
            if t % 32 == 0:
                ps = cpool.tile([128, 1024], f32, tag="cps")
            tr = t % NB
            c0 = (t % 32) * 32
            for g in range(4):
                mm = nc.tensor.matmul(
                    ps[32 * g:32 * g + 32, c0:c0 + 32],
                    st_sb[:, t * 128 + 32 * g:t * 128 + 32 * g + 32],
                    mv_big[:, :, tr * 64 + 16 * g:tr * 64 + 16 * g + 16],
                    start=True, stop=True, skip_group_check=True,
                    tile_position=(0, 32 * g))
            mm_big.append(mm)
            if t % 32 == 31 or t == T_big - 1:
                drain(ps, t // 32,
                      min(1024, (T_big - 32 * (t // 32)) * 32))

        def mm_small_tile(ts):
            nonlocal ps
            if ts % 16 == 0:
                ps = cpool.tile([128, 1024], f32, tag="cps")
            tr = ts % NB
            c0 = (ts % 16) * 64
            for g in range(4):
                mm = nc.tensor.matmul(
                    ps[32 * g:32 * g + 32, c0:c0 + 64],
                    st_sb[:, (T_big + ts) * 128 + 32 * g:
                          (T_big + ts) * 128 + 32 * g + 32],
                    mv_small[:, :, tr * 64 + 16 * g:tr * 64 + 16 * g + 16],
                    start=True, stop=True, skip_group_check=True,
                    tile_position=(0, 32 * g))
            mm_small.append(mm)
            if ts % 16 == 15 or ts == T_small - 1:
                drain(ps, DB_big + ts // 16,
                      min(1024, (T_small - 16 * (ts // 16)) * 64))

        # chunk-pipelined issue: DMA chunk k goes out one iteration before
        # its matmuls, so WAR targets (chunk k-2's matmuls) already exist
        for k in range(nbig_ch + 1):
            if k < nbig_ch:
                dma_big_chunk(k)
            if k >= 1:
                for t in range((k - 1) * CH, min(k * CH, T_big)):
                    mm_big_tile(t)
        for k in range(nsml_ch + 1):
            if k < nsml_ch:
                dma_small_chunk(k)
            if k == 1:
                w_stage(0, U1)   # W half-1 while small DMAs stream
            if k >= 1:
                for ts in range((k - 1) * CH, min(k * CH, T_small)):
                    mm_small_tile(ts)
        w_stage(U1, U)
        nc.vector.tensor_copy(out_sb[:, 0:U], poA[:])
        nc.scalar.copy(out_sb[:, U:2 * U], poB[:])
        nc.sync.dma_start(out_dram[:], out_sb[:])
    nc.compile()
    return nc


def _assemble(outs, segs_per_core, nbig_per_core, T_big, U):
    out = np.zeros((N_NODES, MSG), dtype=np.float32)
    mrow = np.arange(MSG)[None, :]
    for c in range(NCORES):
        segs = segs_per_core[c]
        nbig = nbig_per_core[c]
        P = len(segs)
        if P == 0:
            continue
        po_sb = outs[c].astype(np.float32)           # [128, 2U]
        i = np.arange(P)
        DB_big = -(-T_big // 32)
        p = np.where(i < nbig, i, 256 * DB_big + (i - nbig))  # grid position
        u, g = p // 4, p % 4
        part = 64 * (g % 2)[:, None] + mrow          # [P, 64]
        col = ((g // 2) * U + u)[:, None]
        pos_rows = po_sb[part, col]                  # [P, 64]
        nodes = np.fromiter((segs[k][0] for k in range(P)), dtype=np.int64,
                            count=P)
        np.add.at(out, nodes, pos_rows)
    return out


def kernel(node_features, edge_features, edge_sources, edge_targets,
           hidden, initial, W, b):
    from concourse.bass_utils import run_bass_kernel_spmd

    edge_targets = np.asarray(edge_targets)
    edge_sources = np.asarray(edge_sources)
    edge_features = np.asarray(edge_features, dtype=np.float32)
    hidden = np.asarray(hidden, dtype=np.float32)
    W = np.asarray(W, dtype=np.float32)
    b = np.asarray(b, dtype=np.float32)

    key = edge_targets.tobytes()
    if key in _CACHE:
        layout, nc = _CACHE[key]
    else:
        layout = _build_layout(edge_targets)
        segs_per_core, nbig_per_core, T_big, T_small, U = layout
        nc = _build_program(T_big, T_small, U)
        _CACHE[key] = (layout, nc)
    segs_per_core, nbig_per_core, T_big, T_small, U = layout

    w2 = _build_w2(W)
    in_maps = []
    for c in range(NCORES):
        data = _pack_core(segs_per_core[c], nbig_per_core[c], T_big, T_small,
                          w2, edge_features, edge_sources, hidden)
        in_maps.append({"data": data})

    res = run_bass_kernel_spmd(nc, in_maps, list(range(NCORES)))
    outs = [res.results[c]["out"] for c in range(NCORES)]
    out = _assemble(outs, segs_per_core, nbig_per_core, T_big, U)

    if np.any(b):
        # bias term: out[n] += (sum_{e->n} hidden[src e]) @ Br,
        # Br[h, m] = b[m*16+h].  (b is all-zero for this problem.)
        Br = b.reshape(MSG, HID).T.astype(np.float32)
        acc = np.zeros((N_NODES, HID), dtype=np.float32)
        np.add.at(acc, edge_targets, hidden[edge_sources])
        out += acc @ Br
    return out

